# revision 25
# baseline (speedup 1.0000x reference)
"""GAT (2-layer, PyG-style) Trainium2 Bass kernel, 8-core SPMD.

Sharding: destinations are range-sharded across 8 cores (6250 nodes each).

v2 design (vs v1):
  - node table build is sharded: each core computes 1/8 of the
    h|as rows (49 tiles) and an AllGather assembles the full gather table
    (Shared scratchpad output -> fast HBM-HBM collective).
  - the 0/1 scatter matrices are built ON CHIP from a tiny per-chunk
    dst-local-index column (DLOC) via a broadcasted is_equal against an
    iota row -- no 31MB S01 input, no 31MB/layer HBM load.
  - the per-edge dst attention score is computed on chip:
    PE-transpose of the one-hot chunk + a tiny matmul against the resident
    per-tile dst-score columns, accumulated in PSUM together with the
    gathered src score (identity-lhsT matmul) -> no per-edge dst gather.
  - SWDGE ring enlarged to 4096 descriptors -> 32-chunk gather windows.
  - edge softmax max-subtraction dropped (cancels exactly in the ratio).

Per core: gather per-edge src rows (768B L1 / 256B L2) with SWDGE
dma_gather, compute w = exp(leakyrelu(as[src]+ad[dst])), premultiply rows,
aggregate per dst tile with PSUM-accumulated TensorE matmuls against the
on-chip one-hot blocks (denominators via a w column in the row padding),
bias+ELU, layer-2 node rows, AllGather, repeat aggregation for layer 2.
"""

import math
from dataclasses import dataclass, field

import numpy as np
import ml_dtypes

BF16 = ml_dtypes.bfloat16

P = 128  # partitions / tile edge

# HW bisection: "A"=node tables+allgather, "B1"=+layer1 agg, "C"=+t2 allgather,
# "D"=full
BUILD_STAGE = "D"
USE_SHARED = False  # Shared-output AllGather crashes this platform (NRT 101)


@dataclass
class Cfg:
    n_nodes: int = 50000
    n_edges: int = 800000  # before self loops
    f_in: int = 128
    heads: int = 8
    hid: int = 32
    n_cores: int = 8
    group_tiles: int = 2  # dst tiles per gather group
    split: int = 32768  # int16 index split point
    neg_slope: float = 0.2
    gather_win: int = 8  # chunks per dma_gather call (<= ring/128)

    @property
    def shard(self):
        return self.n_nodes // self.n_cores

    @property
    def hc(self):
        return self.heads * self.hid  # 256

    @property
    def n_tiles(self):
        return math.ceil(self.shard / P)  # dst tiles per core

    @property
    def nt1(self):
        # node-table tiles (total), padded to a multiple of n_cores so the
        # sharded table build covers all rows
        t = math.ceil(self.n_nodes / P)
        return math.ceil(t / self.n_cores) * self.n_cores

    @property
    def nt1_sh(self):
        return self.nt1 // self.n_cores  # node-table tiles per core (49)

    @property
    def n_pad(self):
        return self.nt1 * P

    @property
    def tab_shard(self):
        return self.nt1_sh * P  # 6272

    @property
    def shard_pad(self):
        return self.n_tiles * P


# ------------------------------------------------------------ host preprocess


@dataclass
class Plan:
    """Structure shared by all cores (uniform) + per-core tensor data."""

    CA: list = field(default_factory=list)  # A-chunks per tile (maxed over cores)
    CB: list = field(default_factory=list)
    groups: list = field(default_factory=list)  # per group: list of tile ids
    g_nA: list = field(default_factory=list)
    g_nB: list = field(default_factory=list)
    g_chunk0: list = field(default_factory=list)
    g_tile_of_chunk: list = field(default_factory=list)  # per group: chunk->tile
    k_tot: int = 0
    data: list = field(default_factory=list)  # per-core input arrays


def _wrap16(idx: np.ndarray) -> np.ndarray:
    """[n] -> [128, n/16] int16 gather-index layout (16-wrapped, x8 replicated)."""
    n = idx.shape[0]
    assert n % 16 == 0
    a = idx.astype(np.int16).reshape(n // 16, 16).T  # [16, n/16]
    return np.tile(a, (8, 1)).copy()


def preprocess(edge_index: np.ndarray, cfg: Cfg) -> Plan:
    N = cfg.n_nodes
    loop = np.arange(N, dtype=np.int64)
    src = np.concatenate([edge_index[0].astype(np.int64), loop])
    dst = np.concatenate([edge_index[1].astype(np.int64), loop])

    plan = Plan()
    ncores = cfg.n_cores
    shard = cfg.shard
    cdiv = lambda a, b: -(-a // b)

    per_core = []
    for c in range(ncores):
        m = (dst >= c * shard) & (dst < (c + 1) * shard)
        s_c, d_c = src[m], dst[m] - c * shard
        order = np.argsort(d_c, kind="stable")
        s_c, d_c = s_c[order], d_c[order]
        tiles = []
        for t in range(cfg.n_tiles):
            tm = (d_c >= t * P) & (d_c < (t + 1) * P)
            s_t, d_t = s_c[tm], d_c[tm] - t * P
            a = s_t < cfg.split
            tiles.append((s_t[a], d_t[a], s_t[~a], d_t[~a]))
        per_core.append(tiles)

    for t in range(cfg.n_tiles):
        plan.CA.append(max(cdiv(len(per_core[c][t][0]), P) for c in range(ncores)))
        plan.CB.append(max(cdiv(len(per_core[c][t][2]), P) for c in range(ncores)))

    if cfg.group_tiles == 2:
        # pair big tiles with small ones so group sizes are even
        order = sorted(range(cfg.n_tiles), key=lambda t: plan.CA[t] + plan.CB[t])
        lo, hi = 0, cfg.n_tiles - 1
        while lo < hi:
            plan.groups.append([order[hi], order[lo]])
            lo += 1
            hi -= 1
        if lo == hi:
            plan.groups.append([order[lo]])
    else:
        for g0 in range(0, cfg.n_tiles, cfg.group_tiles):
            plan.groups.append(
                list(range(g0, min(g0 + cfg.group_tiles, cfg.n_tiles)))
            )
    k = 0
    for g in plan.groups:
        plan.g_chunk0.append(k)
        plan.g_nA.append(sum(plan.CA[t] for t in g))
        plan.g_nB.append(sum(plan.CB[t] for t in g))
        k += plan.g_nA[-1] + plan.g_nB[-1]
        t_of = []
        for t in g:
            t_of += [t] * plan.CA[t]
        for t in g:
            t_of += [t] * plan.CB[t]
        plan.g_tile_of_chunk.append(t_of)
    plan.k_tot = k

    for c in range(ncores):
        idxA = []
        dloc = np.full((plan.k_tot, P), -1.0, dtype=np.float32)
        for gi, g in enumerate(plan.groups):
            k0 = plan.g_chunk0[gi]
            nA = plan.g_nA[gi]
            a_off = 0
            b_off = 0
            gA_src, gB_src = [], []
            for t in g:
                sA, dA, sB, dB = per_core[c][t]
                la, lb = plan.CA[t] * P, plan.CB[t] * P
                sA_p = np.concatenate([sA, np.zeros(la - len(sA), np.int64)])
                sB_p = np.concatenate(
                    [sB - cfg.split, np.zeros(lb - len(sB), np.int64)]
                )
                gA_src.append(sA_p)
                gB_src.append(sB_p)
                if len(dA):
                    jj = np.arange(len(dA))
                    dloc[k0 + a_off + jj // P, jj % P] = dA
                if len(dB):
                    jj = np.arange(len(dB))
                    dloc[k0 + nA + b_off + jj // P, jj % P] = dB
                a_off += plan.CA[t]
                b_off += plan.CB[t]
            idxA.append(np.concatenate(gA_src + gB_src))
        cat = lambda xs: (
            np.concatenate([_wrap16(x) for x in xs if len(x)], axis=1)
            if any(len(x) for x in xs)
            else np.zeros((128, 0), np.int16)
        )
        plan.data.append(
            {
                "IDXA": cat(idxA),
                # column k = chunk k's per-edge dst-local index (-1 pads)
                "DLOC": np.ascontiguousarray(dloc.T).astype(BF16),
            }
        )
    return plan


def prep_weights(inputs: dict, cfg: Cfg):
    W1 = np.asarray(inputs["W1"], np.float32)
    a_s1 = np.asarray(inputs["att_src1"], np.float32)
    a_d1 = np.asarray(inputs["att_dst1"], np.float32)
    W2 = np.asarray(inputs["W2"], np.float32)
    a_s2 = np.asarray(inputs["att_src2"], np.float32)
    a_d2 = np.asarray(inputs["att_dst2"], np.float32)
    H, C = cfg.heads, cfg.hid
    W1r = W1.reshape(cfg.f_in, H, C)
    w1as = np.einsum("fhc,hc->fh", W1r, a_s1)
    w1ad = np.einsum("fhc,hc->fh", W1r, a_d1)
    # (c,h)-major message layout: column c*H+h holds head h, channel c
    perm = (np.arange(H * C).reshape(H, C).T).reshape(-1)  # new_col j -> old h*C+c
    W1p = np.concatenate([W1[:, perm], w1as, w1ad], axis=1).astype(BF16)
    w2as = W2 @ a_s2[0]
    w2ad = W2 @ a_d2[0]
    W2p = np.concatenate(
        [W2[perm], w2as[perm, None], w2ad[perm, None]], axis=1
    ).astype(BF16)
    b1 = np.asarray(inputs["b1"], np.float32)[perm]
    b1rep = np.tile(b1[None, :], (P, 1))
    b2rep = np.tile(np.asarray(inputs["b2"], np.float32)[None, :], (P, 1))
    return W1p, W2p, b1rep.astype(np.float32), b2rep.astype(np.float32)


# ---------------------------------------------------------------- bass kernel


def build_kernel(cfg: Cfg, plan: Plan, sim_one_core: bool = False):
    from contextlib import ExitStack

    import concourse.bacc as bacc
    import concourse.mybir as mybir
    import concourse.tile as tile

    fp32 = mybir.dt.float32
    bf16 = mybir.dt.bfloat16
    i16 = mybir.dt.int16
    AF = mybir.ActivationFunctionType
    OP = mybir.AluOpType

    HC = cfg.hc  # 256
    HCX = HC + 16  # 272: [h 256 | as 8 | ad 8]
    H = cfg.heads
    HID = cfg.hid
    NTAB = cfg.n_pad  # 50176
    TSH = cfg.tab_shard  # 6272
    NT_SH = cfg.nt1_sh  # 49
    SH = cfg.shard  # 6250
    T1W = 384  # layer-1 table row width (768B): [h 256 | as 8 | w 8 | junk]
    T2W = 128  # layer-2 table row width (256B): [h2 32 | as2 | ad2 | w | junk]
    NKW2 = HC // P  # 2 chunks for the layer-2 prep matmul
    WIN = cfg.gather_win
    LBATCH = 8  # node tiles per phase-1 load

    nc = bacc.Bacc(
        "TRN2",
        num_devices=1 if sim_one_core else cfg.n_cores,
        num_swdge_queues=1,
        dynamic_dma_scratch_size=16384,
        name="gat8v2",
    )

    xTs = nc.dram_tensor("xTs", [P, TSH], bf16, kind="ExternalInput")
    xTown = nc.dram_tensor("xTown", [P, TSH], bf16, kind="ExternalInput")
    W1p = nc.dram_tensor("W1p", [cfg.f_in, HCX], bf16, kind="ExternalInput")
    W2p = nc.dram_tensor("W2p", [HC, HID + 2], bf16, kind="ExternalInput")
    b1rep = nc.dram_tensor("b1rep", [P, HC], fp32, kind="ExternalInput")
    b2rep = nc.dram_tensor("b2rep", [P, HID], fp32, kind="ExternalInput")
    identity = nc.dram_tensor("identity", [P, P], bf16, kind="ExternalInput")
    iota_in = nc.dram_tensor("iota_in", [P, P], bf16, kind="ExternalInput")
    d0 = plan.data[0]
    IDXA = nc.dram_tensor("IDXA", list(d0["IDXA"].shape), i16, kind="ExternalInput")
    DLOC = nc.dram_tensor("DLOC", [P, plan.k_tot], bf16, kind="ExternalInput")
    OUT = nc.dram_tensor("out", [SH, HID], fp32, kind="ExternalOutput")

    shared_space = "Local" if (sim_one_core or not USE_SHARED) else "Shared"

    with tile.TileContext(nc) as tc, ExitStack() as ctx:
        sb = ctx.enter_context(tc.tile_pool(name="sb", bufs=2))
        sb1 = ctx.enter_context(tc.tile_pool(name="sb1", bufs=1))
        psA = ctx.enter_context(tc.tile_pool(name="psA", bufs=2, space="PSUM"))
        psB = ctx.enter_context(tc.tile_pool(name="psB", bufs=2, space="PSUM"))
        psT = ctx.enter_context(tc.tile_pool(name="psT", bufs=2, space="PSUM"))
        psS = ctx.enter_context(tc.tile_pool(name="psS", bufs=2, space="PSUM"))
        dram = ctx.enter_context(tc.tile_pool(name="dram", bufs=1, space="DRAM"))
        dramS = ctx.enter_context(
            tc.tile_pool(name="dramS", bufs=1, space="DRAM")
        )

        T1sh = dram.tile([TSH, T1W], bf16, tag="T1sh")
        T1x = dramS.tile([NTAB, T1W], bf16, tag="T1x", addr_space=shared_space)
        T2sh = dram.tile([SH, T2W], bf16, tag="T2sh")
        T2full = dramS.tile(
            [cfg.n_nodes, T2W], bf16, tag="T2full", addr_space=shared_space
        )

        # constants / resident tiles
        w1_sb = sb1.tile([cfg.f_in, HCX], bf16, tag="w1")
        nc.sync.dma_start(w1_sb[:], W1p[:])
        w2_sb = sb1.tile([P, NKW2 * (HID + 2)], bf16, tag="w2")
        nc.sync.dma_start(
            w2_sb[:].rearrange("p (a n) -> p a n", a=NKW2),
            W2p[:].rearrange("(a p) n -> p a n", p=P),
        )
        w2_3 = w2_sb[:].rearrange("p (a n) -> p a n", a=NKW2)
        b1_sb = sb1.tile([P, HC], fp32, tag="b1")
        nc.sync.dma_start(b1_sb[:], b1rep[:])
        b2_sb = sb1.tile([P, HID], fp32, tag="b2")
        nc.sync.dma_start(b2_sb[:], b2rep[:])
        id_sb = sb1.tile([P, P], bf16, tag="id")
        nc.sync.dma_start(id_sb[:], identity[:])
        iota_sb = sb1.tile([P, P], bf16, tag="iota")
        nc.sync.dma_start(iota_sb[:], iota_in[:])
        dloc_sb = sb1.tile([P, plan.k_tot], bf16, tag="dloc")
        nc.sync.dma_start(dloc_sb[:], DLOC[:])
        # resident per-tile dst scores: layer 1 ad [128, 49*8], layer 2
        # [as2|ad2] [128, 49*2] (as2 kept only to simplify the epilogue copy)
        sc1own = sb1.tile([P, NT_SH * H], bf16, tag="sc1own")
        sc2own = sb1.tile([P, NT_SH], bf16, tag="sc2own")

        # ---------------- phase 1: node table (sharded 8x) ----------------
        for w0 in range(0, NT_SH, LBATCH):
            wn = min(LBATCH, NT_SH - w0)
            xt = sb.tile([P, LBATCH * P], bf16, tag="xt", bufs=3)
            nc.sync.dma_start(xt[:, : wn * P], xTs[:, w0 * P : (w0 + wn) * P])
            for j in range(wn):
                pt = psA.tile([P, HCX], fp32, tag="pagg")
                nc.tensor.matmul(
                    out=pt[:, : HC + H],
                    lhsT=xt[:, j * P : (j + 1) * P],
                    rhs=w1_sb[:, : HC + H],
                    start=True,
                    stop=True,
                )
                stg = sb.tile([P, HC + H], bf16, tag="stg1", bufs=3)
                if j % 2 == 0:
                    nc.vector.tensor_copy(stg[:], pt[:, : HC + H])
                else:
                    nc.scalar.copy(stg[:], pt[:, : HC + H])
                i = w0 + j
                nc.sync.dma_start(T1sh[i * P : (i + 1) * P, : HC + H], stg[:])

        # assemble the full gather table
        if sim_one_core:
            for c in range(cfg.n_cores):
                nc.sync.dma_start(T1x[c * TSH : (c + 1) * TSH, :], T1sh[:, :])
        else:
            nc.gpsimd.collective_compute(
                "AllGather",
                OP.bypass,
                replica_groups=[list(range(cfg.n_cores))],
                ins=[T1sh.opt()],
                outs=[T1x.opt()],
            )

        # phase 1b: own-dst-shard ad scores, kept resident in SBUF
        for w0 in range(0, NT_SH, LBATCH):
            wn = min(LBATCH, NT_SH - w0)
            xo = sb.tile([P, LBATCH * P], bf16, tag="xo", bufs=2)
            nc.sync.dma_start(xo[:, : wn * P], xTown[:, w0 * P : (w0 + wn) * P])
            for j in range(wn):
                t = w0 + j
                pt = psB.tile([P, HID + 2], fp32, tag="p2")
                nc.tensor.matmul(
                    out=pt[:, :H],
                    lhsT=xo[:, j * P : (j + 1) * P],
                    rhs=w1_sb[:, HC + H : HCX],
                    start=True,
                    stop=True,
                )
                nc.scalar.copy(sc1own[:, t * H : (t + 1) * H], pt[:, :H])

        # ------------- layer-1 per-tile epilogue: bias, ELU, layer-2 rows ----
        def epilogue1(t, o_f):
            # elu(y) = relu(y) + min(exp(y), 1) - 1   (y values are small
            # enough that exp(y) stays finite in fp32)
            y = sb.tile([P, HC], fp32, tag="ep_y")
            nc.vector.tensor_tensor(out=y[:], in0=o_f[:], in1=b1_sb[:], op=OP.add)
            ex = sb.tile([P, HC], fp32, tag="ep_ex")
            nc.scalar.activation(ex[:], y[:], AF.Exp)
            nc.vector.tensor_scalar(
                out=ex[:], in0=ex[:], scalar1=1.0, scalar2=1.0,
                op0=OP.min, op1=OP.subtract,
            )
            nc.vector.tensor_scalar_max(y[:], y[:], 0.0)  # relu, in place
            elu_bf = sb.tile([P, HC], bf16, tag="ep_bf")
            nc.vector.tensor_tensor(out=elu_bf[:], in0=y[:], in1=ex[:], op=OP.add)
            eluT = sb.tile([P, HC], bf16, tag="ep_eT")
            for j in range(NKW2):
                ptT = psT.tile([P, P], bf16, tag="ptT")
                nc.tensor.transpose(
                    out=ptT[:], in_=elu_bf[:, j * P : (j + 1) * P], identity=id_sb[:]
                )
                nc.scalar.copy(eluT[:, j * P : (j + 1) * P], ptT[:])
            p2 = psB.tile([P, HID + 2], fp32, tag="p2")
            for j in range(NKW2):
                nc.tensor.matmul(
                    out=p2[:],
                    lhsT=eluT[:, j * P : (j + 1) * P],
                    rhs=w2_3[:, j, :],
                    start=(j == 0),
                    stop=(j == NKW2 - 1),
                )
            r2 = sb.tile([P, HID + 2], bf16, tag="r2")
            nc.scalar.copy(r2[:], p2[:])
            nc.vector.tensor_copy(sc2own[:, t : t + 1], r2[:, HID + 1 : HID + 2])
            rows = min(SH - t * P, P)
            nc.sync.dma_start(T2sh[t * P : t * P + rows, : HID + 2], r2[:rows, :])

        # ---------------- shared gather/aggregate phase ----------------
        maxng = max(a + b for a, b in zip(plan.g_nA, plan.g_nB))
        TB = 8  # transposes batched per PSUM tile

        def agg_layer(layer):
            elem_h = T1W if layer == 1 else T2W
            nhead = H if layer == 1 else 1
            rhs_w = HC if layer == 1 else HID  # message width
            as_col = HC if layer == 1 else HID  # as column in gathered rows
            wcol = (HC + H) if layer == 1 else (HID + 2)  # w column
            rw = wcol + nhead  # matmul rhs width incl junk + w cols
            tabh = T1x if layer == 1 else T2full

            def st_gather(gi):
                """issue the src-row gathers for group gi"""
                nA, nBg = plan.g_nA[gi], plan.g_nB[gi]
                ng = nA + nBg
                k0 = plan.g_chunk0[gi]
                bufh = sb.tile([P, maxng * elem_h], bf16, tag="bufh", bufs=3)
                bufh3 = bufh[:, : ng * elem_h].rearrange("p (k e) -> p k e", e=elem_h)
                idx_sb = sb.tile([P, maxng * 8], i16, tag="idxg", bufs=3)
                nc.sync.dma_start(
                    idx_sb[:, : ng * 8], IDXA[:, k0 * 8 : (k0 + ng) * 8]
                )

                def win_gather(c0, n_chunks, table_ap):
                    for w0 in range(0, n_chunks, WIN):
                        wn = min(WIN, n_chunks - w0)
                        nc.gpsimd.dma_gather(
                            out_ap=bufh3[:, c0 + w0 : c0 + w0 + wn, :],
                            in_ap=table_ap,
                            idxs_ap=idx_sb[:, (c0 + w0) * 8 : (c0 + w0 + wn) * 8],
                            num_idxs=wn * P,
                            num_idxs_reg=wn * P,
                            elem_size=elem_h,
                            queue_num=0,
                        )

                if nA:
                    win_gather(0, nA, tabh[:, :])
                if nBg:
                    win_gather(nA, nBg, tabh[cfg.split :, :])
                return bufh3

            def st_prep(gi):
                """one-hot blocks + transposed one-hots + dst-score projection
                (independent of the gathers / the node tables)"""
                nA, nBg = plan.g_nA[gi], plan.g_nB[gi]
                ng = nA + nBg
                k0 = plan.g_chunk0[gi]
                t_of = plan.g_tile_of_chunk[gi]

                # S01[e, k, d] = (dloc[e, k] == d)
                s01 = sb.tile([P, maxng * P], bf16, tag="s01", bufs=3)
                s01_3 = s01[:, : ng * P].rearrange("p (k d) -> p k d", d=P)
                nc.vector.tensor_tensor(
                    out=s01_3,
                    in0=dloc_sb[:, k0 : k0 + ng].to_broadcast([P, ng, P]),
                    in1=iota_sb[:]
                    .rearrange("p (o d) -> p o d", o=1)
                    .to_broadcast([P, ng, P]),
                    op=OP.is_equal,
                )

                ps_sc = psS.tile([P, maxng * H], fp32, tag="ps_sc")
                n_b = -(-ng // TB)
                s01T_t = {}

                def admats(b0):
                    bn = min(TB, ng - b0)
                    s01T = s01T_t.pop(b0)
                    for j in range(bn):
                        k = b0 + j
                        t = t_of[k]
                        adt = (
                            sc1own[:, t * H : (t + 1) * H]
                            if layer == 1
                            else sc2own[:, t : t + 1]
                        )
                        nc.tensor.matmul(
                            out=ps_sc[:, k * nhead : (k + 1) * nhead],
                            lhsT=s01T[:, j * P : (j + 1) * P],
                            rhs=adt,
                            start=True,
                            stop=True,
                        )

                for bi in range(n_b):
                    b0 = bi * TB
                    bn = min(TB, ng - b0)
                    ptT = psT.tile([P, TB * P], bf16, tag="ptT")
                    for j in range(bn):
                        nc.tensor.transpose(
                            out=ptT[:, j * P : (j + 1) * P],
                            in_=s01_3[:, b0 + j, :],
                            identity=id_sb[:],
                        )
                    s01T = sb.tile([P, TB * P], bf16, tag="s01T", bufs=3)
                    nc.scalar.copy(s01T[:, : bn * P], ptT[:, : bn * P])
                    s01T_t[b0] = s01T
                    if bi > 0:
                        admats((bi - 1) * TB)
                admats((n_b - 1) * TB)
                # free the PSUM for the next group's projection
                s_ad = sb.tile([P, maxng * H], fp32, tag="s_ad", bufs=3)
                nc.scalar.copy(s_ad[:, : ng * nhead], ps_sc[:, : ng * nhead])
                return s01_3, s_ad

            def st_wchain(gi, bufh3, s_ad):
                """per-edge scores -> w -> premultiply (needs the gathers)"""
                nA, nBg = plan.g_nA[gi], plan.g_nB[gi]
                ng = nA + nBg
                # s = ad[dst] + as[src];  w = exp(max(s, 0.2 s))
                s_f = sb.tile([P, maxng * H], fp32, tag="s_f", bufs=2)
                nc.vector.tensor_tensor(
                    out=s_f[:, : ng * nhead].rearrange("p (k h) -> p k h", h=nhead),
                    in0=s_ad[:, : ng * nhead].rearrange("p (k h) -> p k h", h=nhead),
                    in1=bufh3[:, :, as_col : as_col + nhead],
                    op=OP.add,
                )
                s_lr = sb.tile([P, maxng * H], fp32, tag="s_lr", bufs=2)
                nc.scalar.mul(s_lr[:, : ng * nhead], s_f[:, : ng * nhead], cfg.neg_slope)
                nc.vector.tensor_tensor(
                    out=s_lr[:, : ng * nhead],
                    in0=s_lr[:, : ng * nhead],
                    in1=s_f[:, : ng * nhead],
                    op=OP.max,
                )
                w_bf = sb.tile([P, maxng * H], bf16, tag="w_bf", bufs=2)
                nc.scalar.activation(w_bf[:, : ng * nhead], s_lr[:, : ng * nhead], AF.Exp)
                w3 = w_bf[:, : ng * nhead].rearrange("p (k h) -> p k h", h=nhead)
                # w into the padding column(s) -> softmax denominators
                nc.scalar.copy(bufh3[:, :, wcol : wcol + nhead], w3)

                # premultiply gathered message rows by w (in place);
                # split across DVE and Pool to balance engine load
                if layer == 1:
                    CSPL = 17  # channel split: DVE below, Pool above
                    mw = bufh3[:, :, :HC].rearrange("p k (c h) -> p k c h", h=H)
                    wb = w_bf[:, : ng * H].rearrange(
                        "p (k o h) -> p k o h", o=1, h=H
                    ).to_broadcast([P, ng, HID, H])
                    nc.vector.tensor_tensor(
                        out=mw[:, :, :CSPL, :],
                        in0=mw[:, :, :CSPL, :],
                        in1=wb[:, :, :CSPL, :],
                        op=OP.mult,
                    )
                    nc.gpsimd.tensor_tensor(
                        out=mw[:, :, CSPL:, :],
                        in0=mw[:, :, CSPL:, :],
                        in1=wb[:, :, CSPL:, :],
                        op=OP.mult,
                    )
                else:
                    mw = bufh3[:, :, :HID]
                    wb = w_bf[:, :ng].to_broadcast([P, ng, HID])
                    nc.vector.tensor_tensor(out=mw, in0=mw, in1=wb, op=OP.mult)

            def st_agg(gi, bufh3, s01_3):
                """per-dst-tile aggregation + epilogue"""
                nA = plan.g_nA[gi]
                a_off, b_off = 0, 0
                for t in plan.groups[gi]:
                    pt_full = psA.tile([P, HCX], fp32, tag="pagg")
                    pt = pt_full[:, :rw]
                    chunks = [a_off + j for j in range(plan.CA[t])] + [
                        nA + b_off + j for j in range(plan.CB[t])
                    ]
                    nk = len(chunks)
                    for ci, k in enumerate(chunks):
                        nc.tensor.matmul(
                            out=pt[:],
                            lhsT=s01_3[:, k, :],
                            rhs=bufh3[:, k, :rw],
                            start=(ci == 0),
                            stop=(ci == nk - 1),
                        )
                    a_off += plan.CA[t]
                    b_off += plan.CB[t]

                    den = sb.tile([P, nhead], fp32, tag="den")
                    nc.vector.tensor_scalar_max(
                        den[:], pt[:, wcol : wcol + nhead], 1e-20
                    )
                    den_r = sb.tile([P, nhead], fp32, tag="denr")
                    nc.vector.reciprocal(den_r[:], den[:])
                    o_f = sb.tile([P, rhs_w], fp32, tag="o_f")
                    if layer == 1:
                        nc.vector.tensor_tensor(
                            out=o_f[:].rearrange("p (c h) -> p c h", h=nhead),
                            in0=pt[:, :rhs_w].rearrange("p (c h) -> p c h", h=nhead),
                            in1=den_r[:]
                            .rearrange("p (o h) -> p o h", o=1)
                            .to_broadcast([P, HID, nhead]),
                            op=OP.mult,
                        )
                        epilogue1(t, o_f)
                    else:
                        nc.scalar.activation(
                            o_f[:], pt[:, :HID], AF.Copy, scale=den_r[:]
                        )
                        nc.vector.tensor_tensor(
                            out=o_f[:], in0=o_f[:], in1=b2_sb[:], op=OP.add
                        )
                        rows = min(SH - t * P, P)
                        nc.sync.dma_start(OUT[t * P : t * P + rows, :], o_f[:rows, :])

            # software pipeline: gather(i) + prep(i) | wchain(i-1) | agg(i-2)
            # prep is gather-independent, so the early iterations overlap the
            # table AllGather that the first gathers must wait for
            n_g = len(plan.groups)
            st = {}
            for gi in range(n_g + 2):
                if gi < n_g:
                    bufh3 = st_gather(gi)
                    s01_3, s_ad = st_prep(gi)
                    st[gi] = [bufh3, s01_3, s_ad]
                if 0 <= gi - 1 < n_g:
                    st_wchain(gi - 1, st[gi - 1][0], st[gi - 1][2])
                if 0 <= gi - 2:
                    st_agg(gi - 2, st[gi - 2][0], st[gi - 2][1])
                    del st[gi - 2]

        if BUILD_STAGE != "A":
            agg_layer(1)

        if BUILD_STAGE in ("A", "B1"):
            stgx = sb.tile([P, HID], fp32, tag="dumm")
            for t in range(cfg.n_tiles):
                rows = min(SH - t * P, P)
                nc.vector.tensor_copy(stgx[:rows, :], b2_sb[:rows, :])
                nc.sync.dma_start(OUT[t * P : t * P + rows, :], stgx[:rows, :])

        if BUILD_STAGE in ("C", "D"):
            if sim_one_core:
                for c in range(cfg.n_cores):
                    nc.sync.dma_start(T2full[c * SH : (c + 1) * SH, :], T2sh[:, :])
            else:
                nc.gpsimd.collective_compute(
                    "AllGather",
                    OP.bypass,
                    replica_groups=[list(range(cfg.n_cores))],
                    ins=[T2sh.opt()],
                    outs=[T2full.opt()],
                )

        if BUILD_STAGE == "D":
            agg_layer(2)
        elif BUILD_STAGE == "C":
            stg0 = sb.tile([P, HID], fp32, tag="dumm")
            for t in range(cfg.n_tiles):
                rows = min(SH - t * P, P)
                nc.vector.tensor_copy(stg0[:rows, :], b2_sb[:rows, :])
                nc.sync.dma_start(OUT[t * P : t * P + rows, :], stg0[:rows, :])

    nc.compile()
    return nc


# -------------------------------------------------------------------- driver


def make_in_maps(inputs: dict, cfg: Cfg, plan: Plan):
    x = np.asarray(inputs["x"], np.float32)
    W1p, W2p, b1rep, b2rep = prep_weights(inputs, cfg)
    x_pad = np.zeros((cfg.n_pad, cfg.f_in), np.float32)
    x_pad[: cfg.n_nodes] = x
    ident = np.eye(P, dtype=BF16)
    iota = np.tile(np.arange(P, dtype=np.float32)[None, :], (P, 1)).astype(BF16)
    in_maps = []
    for c in range(cfg.n_cores):
        xs = x_pad[c * cfg.tab_shard : (c + 1) * cfg.tab_shard]
        xo = np.zeros((cfg.tab_shard, cfg.f_in), np.float32)
        xo[: cfg.shard] = x[c * cfg.shard : (c + 1) * cfg.shard]
        d = plan.data[c]
        in_maps.append(
            {
                "xTs": np.ascontiguousarray(xs.T).astype(BF16),
                "xTown": np.ascontiguousarray(xo.T).astype(BF16),
                "W1p": W1p,
                "W2p": W2p,
                "b1rep": b1rep,
                "b2rep": b2rep,
                "identity": ident,
                "iota_in": iota,
                "IDXA": d["IDXA"],
                "DLOC": d["DLOC"],
            }
        )
    return in_maps


def kernel(**inputs) -> np.ndarray:
    cfg = Cfg()
    edge_index = np.asarray(inputs["edge_index"])
    plan = preprocess(edge_index, cfg)
    in_maps = make_in_maps(inputs, cfg, plan)
    nc = build_kernel(cfg, plan)

    from concourse.bass_utils import run_bass_kernel_spmd

    res = run_bass_kernel_spmd(nc, in_maps, core_ids=list(range(cfg.n_cores)))
    out = np.concatenate([r["out"] for r in res.results], axis=0)
    return np.ascontiguousarray(out).astype(np.float32)
